# revision 1
# baseline (speedup 1.0000x reference)
"""Causal self-attention Trainium2 kernel (8-core head-parallel).

Full inputs in, full output out. Sharding strategy:
  - 16 heads / 8 cores -> 2 heads per core, both batch elems (4 (b,h) attention
    problems per core).
  - QKV projection column-parallel: each core gets w_attn[:, cols-of-its-heads]
    as a [1024, 384] slice (q 128 | k 128 | v 128), q pre-scaled by 1/sqrt(D).
  - c_proj row-parallel: each core gets w_proj[128c:128c+128, :] and produces a
    partial [B, C, T] output (transposed); host sums the 8 partials (the
    all-reduce of the row-parallel projection), transposes back, adds bias.

On-device layout (per core) keeps everything transposed to avoid transposes:
  xT [C=1024, TOK=4096] bf16 (host-pretransposed)
  qkv^T = Wslice^T @ xT  -> q^T,k^T [128(2 heads x 64), TOK], v^T [128, TOK]
  v^T is PE-transposed into V tiles [128 s, 65] with a ones column (col 64),
  so the AV matmul also produces the softmax denominator Z as row 64.
  S^T[s, q] = k^T-tile (stationary) x q^T (moving); exp on ACT with bias -4;
  causal handled by only computing q >= s-tile-start plus one [128,128]
  triangular mask multiply per diagonal tile (on GPSIMD).
  y_aug^T [65, q] accumulates over s-tiles in PSUM; normalization multiplies by
  a PE-broadcast of 1/Z; out^T[b] = wp_slice^T @ y^T done per 128-col tile.
"""

import math

import numpy as np
import ml_dtypes

import concourse.bass as bass
from concourse import bacc
import concourse.mybir as mybir
from concourse.tile import TileContext
from concourse.bass_utils import run_bass_kernel_spmd

BF16 = mybir.dt.bfloat16
F32 = mybir.dt.float32
NPBF16 = ml_dtypes.bfloat16

P = 128
B, T, C = 2, 2048, 1024
H, D = 16, 64
NCORES = 8
HPC = H // NCORES          # heads per core
TOK = B * T                # 4096 flattened tokens (b-major)
NCT = C // P               # 8 contraction tiles for the projections
NTC = TOK // 512           # 8 token chunks of 512
QW = 1024                  # q window width for attention inner loop
EXP_BIAS = -4.0            # exp(s - 4): cancels in normalization, guards tail


def _patch_act_tables():
    """Force every exp/ln activation onto the single table set that contains
    both, so the kernel never pays mid-stream ACT_TABLE_LOAD switches."""
    import concourse.bacc as bacc_mod
    if getattr(bacc_mod, "_act_tables_patched", False):
        return
    orig = bacc_mod.get_activation_tables
    EXP = mybir.ActivationFunctionType.Exp
    LN = mybir.ActivationFunctionType.Ln

    def patched(arch):
        t = orig(arch)
        if any(EXP in f and LN in f for f in t.values()):
            for name, fns in t.items():
                if "natural_log_exp" not in name and (EXP in fns or LN in fns):
                    t[name] = fns - {EXP, LN}
        return t

    bacc_mod.get_activation_tables = patched
    bacc_mod._act_tables_patched = True


def build_nc(with_bias: bool) -> bacc.Bacc:
    _patch_act_tables()
    nc = bacc.Bacc(None, target_bir_lowering=False)

    xt = nc.dram_tensor("xt", [C, TOK], BF16, kind="ExternalInput")
    wqkv = nc.dram_tensor("wqkv", [C, 3 * P], BF16, kind="ExternalInput")
    wp = nc.dram_tensor("wp", [P, C], BF16, kind="ExternalInput")
    tri = nc.dram_tensor("tri", [P, P], BF16, kind="ExternalInput")
    ident = nc.dram_tensor("ident", [P, P], BF16, kind="ExternalInput")
    ones64 = nc.dram_tensor("ones64", [1, 64], BF16, kind="ExternalInput")
    if with_bias:
        bqkv = nc.dram_tensor("bqkv", [1, 3 * P], BF16, kind="ExternalInput")
        ones512 = nc.dram_tensor("ones512", [1, 512], BF16, kind="ExternalInput")
    outT = nc.dram_tensor("outT", [B, C, T], BF16, kind="ExternalOutput")

    EXP = mybir.ActivationFunctionType.Exp
    LOG = mybir.ActivationFunctionType.Ln

    with TileContext(nc) as tc:
        with (
            tc.tile_pool(name="consts", bufs=1) as consts,
            tc.tile_pool(name="px", bufs=1) as px,
            tc.tile_pool(name="pqkv", bufs=1) as pqkv,
            tc.tile_pool(name="py", bufs=1) as py,
            tc.tile_pool(name="pwork", bufs=2) as pwork,
        ):
            # ---- constant / input loads ----
            wqkv_sb = consts.tile([P, NCT, 3 * P], BF16)
            for ct in range(NCT):
                nc.sync.dma_start(wqkv_sb[:, ct, :], wqkv[ct * P:(ct + 1) * P, :])
            wp_sb = consts.tile([P, C], BF16)
            nc.sync.dma_start(wp_sb, wp[:, :])
            tri_sb = consts.tile([P, P], BF16)
            nc.sync.dma_start(tri_sb, tri[:, :])
            ident_sb = consts.tile([P, P], BF16)
            nc.sync.dma_start(ident_sb, ident[:, :])
            ones64_sb = consts.tile([1, 64], BF16)
            nc.sync.dma_start(ones64_sb, ones64[:, :])
            expb = consts.tile([P, 1], F32)
            nc.vector.memset(expb, EXP_BIAS)
            if with_bias:
                bqkv_sb = consts.tile([1, 3 * P], BF16)
                nc.sync.dma_start(bqkv_sb, bqkv[:, :])
                ones512_sb = consts.tile([1, 512], BF16)
                nc.sync.dma_start(ones512_sb, ones512[:, :])

            xt_sb = px.tile([P, NCT, TOK], BF16)
            for half in range(2):
                lo, hi = half * T, (half + 1) * T
                for ct in range(NCT):
                    nc.sync.dma_start(xt_sb[:, ct, lo:hi],
                                      xt[ct * P:(ct + 1) * P, lo:hi])

            # qk^T slabs: ft=0 -> q^T (rows: h0 0:64, h1 64:128), ft=1 -> k^T
            qkT = pqkv.tile([P, 2, TOK], BF16)
            vT = pqkv.tile([P, TOK], BF16)
            # V tiles [s, d] with ones column at 64; index = (b*HPC+h)*T/P + st
            V = pqkv.tile([P, B * HPC * (T // P), 65], BF16)
            nc.vector.memset(V, 1.0)  # pre-fill so col 64 is the ones column
            yT = py.tile([P, B, T], BF16)

            # ---- phase 1: QKV projection (q^T, k^T, v^T) ----
            with (
                tc.tile_pool(name="ps_qkv", bufs=5, space="PSUM") as ps_qkv,
                tc.tile_pool(name="ps_t", bufs=2, space="PSUM") as ps_t,
            ):
                # HAM warm-up: keep the PE streaming dummy matmuls while the
                # xt DMA lands, so the clock gate opens before real work.
                scratch = ps_qkv.tile([P, 512], F32, tag="warm", bufs=1)

                def dummy_mms(n):
                    for _ in range(n):
                        nc.tensor.matmul(
                            scratch[:, 0:384],
                            wqkv_sb[:, 0, 0:P],
                            wqkv_sb[:, 0, :],
                            start=True, stop=True,
                        )

                dummy_mms(12)
                NG = 2  # token-chunks per psum group (full double buffering)
                for ft in range(3):
                    for tcq in range(2):  # b0 token chunks only; b1 is
                                          # interleaved into attention below
                        psums = []
                        for q in range(NG):
                            pt = ps_qkv.tile([P, 512], F32, tag="qkv", name=f"qkvps_{ft}_{tcq}_{q}")
                            psums.append(pt)
                        for ct in range(NCT):
                            for q in range(NG):
                                tcc = tcq * NG + q
                                nc.tensor.matmul(
                                    psums[q],
                                    wqkv_sb[:, ct, ft * P:(ft + 1) * P],
                                    xt_sb[:, ct, tcc * 512:(tcc + 1) * 512],
                                    start=(ct == 0),
                                    stop=(ct == NCT - 1 and not with_bias),
                                )
                            if ft == 0 and tcq == 0:
                                # first pass is paced by the xt DMA stream;
                                # dummies keep the PE from idling cold
                                dummy_mms(8)
                            elif ft == 0 and tcq == 1:
                                dummy_mms(3)
                        if with_bias:
                            for q in range(NG):
                                nc.tensor.matmul(
                                    psums[q],
                                    bqkv_sb[0:1, ft * P:(ft + 1) * P],
                                    ones512_sb[0:1, :],
                                    start=False,
                                    stop=True,
                                )
                        for q in range(NG):
                            tcc = tcq * NG + q
                            if ft < 2:
                                nc.scalar.copy(
                                    qkT[:, ft, tcc * 512:(tcc + 1) * 512], psums[q]
                                )
                            else:
                                nc.scalar.copy(
                                    vT[:, tcc * 512:(tcc + 1) * 512], psums[q]
                                )

                # v^T -> V tiles for b0 (PE transpose, both heads at once)
                for st in range(T // P):
                    pt = ps_t.tile([P, P], BF16, tag="vt", name=f"vtps_0_{st}")
                    nc.tensor.transpose(
                        pt, vT[:, st * P:(st + 1) * P], ident_sb
                    )
                    for h in range(HPC):
                        vidx = h * (T // P) + st
                        nc.vector.tensor_copy(
                            V[:, vidx, 0:64], pt[:, h * 64:(h + 1) * 64]
                        )

            # ---- phase 2+3: attention with interleaved normalize + proj ----
            # Normalize of window i is emitted inside window i+1's st loop, and
            # projection units are sprinkled into later windows, so the PE
            # never idles long enough for HAM to re-throttle.
            with tc.tile_pool(name="ps_att", bufs=1, space="PSUM") as ps_att:
                proj_ready = []
                qkv_b1 = []

                def qkv_b1_unit(ft, tcc):
                    pt = ps_att.tile([P, QW], F32, tag="att", bufs=4,
                                     name=f"qb1_{ft}_{tcc}")
                    for ct in range(NCT):
                        nc.tensor.matmul(
                            pt[:, 0:512],
                            wqkv_sb[:, ct, ft * P:(ft + 1) * P],
                            xt_sb[:, ct, tcc * 512:(tcc + 1) * 512],
                            start=(ct == 0),
                            stop=(ct == NCT - 1 and not with_bias),
                        )
                    if with_bias:
                        nc.tensor.matmul(
                            pt[:, 0:512],
                            bqkv_sb[0:1, ft * P:(ft + 1) * P],
                            ones512_sb[0:1, :],
                            start=False, stop=True,
                        )
                    if ft < 2:
                        nc.vector.tensor_copy(
                            qkT[:, ft, tcc * 512:(tcc + 1) * 512], pt[:, 0:512])
                    else:
                        nc.vector.tensor_copy(
                            vT[:, tcc * 512:(tcc + 1) * 512], pt[:, 0:512])

                def vtrans_b1_unit(st):
                    pt2 = ps_att.tile([P, P], BF16, tag="att", bufs=4,
                                      name=f"vtps_1_{st}")
                    nc.tensor.transpose(
                        pt2, vT[:, T + st * P: T + (st + 1) * P], ident_sb)
                    for h in range(HPC):
                        vidx = (HPC + h) * (T // P) + st
                        nc.vector.tensor_copy(
                            V[:, vidx, 0:64], pt2[:, h * 64:(h + 1) * 64])

                for ft in (2, 0, 1):
                    for tcc in range(4, 8):
                        qkv_b1.append((qkv_b1_unit, ft, tcc))
                for st in range(T // P):
                    qkv_b1.append((vtrans_b1_unit, st))

                def pop_qkv_b1(n):
                    for _ in range(min(n, len(qkv_b1))):
                        f, *a = qkv_b1.pop(0)
                        f(*a)

                def emit_proj_unit(b, of, tcc):
                    po = ps_att.tile([P, QW], F32, tag="att", bufs=4,
                                     name=f"ops_{b}_{of}_{tcc}")
                    nc.tensor.matmul(
                        po[:, 0:512],
                        wp_sb[:, of * P:(of + 1) * P],
                        yT[:, b, tcc * 512:(tcc + 1) * 512],
                        start=True, stop=True,
                    )
                    ot = pwork.tile([P, 512], BF16, tag="ot", bufs=6,
                                    name=f"ot_{b}_{of}_{tcc}")
                    nc.vector.tensor_copy(ot, po[:, 0:512])
                    nc.sync.dma_start(
                        outT[b, of * P:(of + 1) * P,
                             tcc * 512:(tcc + 1) * 512],
                        ot,
                    )

                def sprinkle_proj(n):
                    for _ in range(min(n, len(proj_ready))):
                        emit_proj_unit(*proj_ready.pop(0))

                def norm_start(b, qh, ys):
                    """Emit the y-psum readers now (frees the slots fast);
                    return deferred closures for the rest of the chain."""
                    qbase = qh * QW
                    tails = []
                    for h in range(HPC):
                        lz = pwork.tile([1, QW], F32, tag="lz", bufs=4,
                                        name=f"lz_{b}_{qh}_{h}")
                        nc.scalar.activation(lz, ys[h][64:65, :], LOG)
                        ynum = pwork.tile([64, QW], BF16, tag="ynum", bufs=4,
                                          name=f"ynum_{b}_{qh}_{h}")
                        nc.vector.tensor_copy(ynum, ys[h][0:64, :])

                        def tail(h=h, lz=lz, ynum=ynum):
                            r = pwork.tile([1, QW], BF16, tag="r", bufs=4,
                                           name=f"r_{b}_{qh}_{h}")
                            nc.scalar.activation(r, lz, EXP, scale=-1.0)
                            rb = ps_att.tile([P, QW], F32, tag="att", bufs=4,
                                             name=f"rb_{b}_{qh}_{h}")
                            for c0 in range(0, QW, 512):
                                nc.tensor.matmul(
                                    rb[0:64, c0:c0 + 512],
                                    ones64_sb,
                                    r[:, c0:c0 + 512],
                                    start=True, stop=True,
                                )
                            nc.vector.tensor_mul(
                                yT[h * 64:(h + 1) * 64, b, qbase:qbase + QW],
                                ynum,
                                rb[0:64, :],
                            )
                            if h == HPC - 1:
                                proj_ready.extend(
                                    (b, of, 2 * qh + j)
                                    for j in range(2) for of in range(NCT)
                                )
                        tails.append(tail)
                    return tails

                pending_tails = []
                for b in range(B):
                    for qh in range(T // QW):
                        if b == 1 and qh == 0:
                            pop_qkv_b1(len(qkv_b1))  # flush before b1 windows
                        qbase = qh * QW
                        n_st = (qbase + QW) // P
                        # lazy y allocation: let the first S tiles of the
                        # window grab the promptly-freed psum slots
                        ys = {}

                        def get_y(h, b=b, qh=qh, ys=ys):
                            if h not in ys:
                                ys[h] = ps_att.tile(
                                    [P, QW], F32, tag="att", bufs=4,
                                    name=f"yps_{b}_{qh}_{h}")
                            return ys[h]

                        # last s-tile contributing to each 512-wide output bank
                        last_st = {0: (qbase + 512) // P - 1, 1: n_st - 1}
                        av_fifo = []
                        for st in range(n_st):
                            s0 = st * P
                            qa = max(qbase, s0)          # global q start
                            w = qbase + QW - qa          # active width
                            for h in range(HPC):
                                ps = ps_att.tile([P, QW], F32, tag="att", bufs=4,
                                                 name=f"sps_{b}_{qh}_{st}_{h}")
                                n_beat = 6 if (st == 0 and h == 0) else 0
                                for _ in range(n_beat):
                                    # heartbeat: keep the PE clock gate open
                                    # through pipeline bubbles
                                    nc.tensor.matmul(
                                        ps[:, 0:384],
                                        wqkv_sb[:, 0, 0:P],
                                        wqkv_sb[:, 0, :],
                                        start=True, stop=True,
                                    )
                                for c0 in range(0, w, 512):
                                    cw = min(512, w - c0)
                                    nc.tensor.matmul(
                                        ps[:, c0:c0 + cw],
                                        qkT[h * 64:(h + 1) * 64, 1,
                                            b * T + s0: b * T + s0 + P],
                                        qkT[h * 64:(h + 1) * 64, 0,
                                            b * T + qa + c0: b * T + qa + c0 + cw],
                                        start=True, stop=True,
                                    )
                                es = pwork.tile([P, QW], BF16, tag="expS", bufs=8,
                                                name=f"es_{b}_{qh}_{st}_{h}")
                                nc.scalar.activation(
                                    es[:, 0:w], ps[:, 0:w], EXP, bias=expb
                                )
                                if s0 >= qbase:
                                    nc.gpsimd.tensor_mul(
                                        es[:, 0:P], es[:, 0:P], tri_sb
                                    )

                                def av(st=st, h=h, es=es, off=qa - qbase,
                                       vidx=(b * HPC + h) * (T // P) + st):
                                    ysh = get_y(h)
                                    for k in range(2):
                                        lo = max(off, k * 512)
                                        hi = (k + 1) * 512
                                        if lo >= hi:
                                            continue
                                        nc.tensor.matmul(
                                            ysh[0:65, lo:hi],
                                            V[:, vidx, :],
                                            es[:, lo - off:hi - off],
                                            start=(st == 0),
                                            stop=(st == last_st[k]),
                                        )
                                av_fifo.append(av)
                                if len(av_fifo) > 4:
                                    av_fifo.pop(0)()
                            if pending_tails and st >= 1:
                                pending_tails.pop(0)()
                            if st >= 2:
                                sprinkle_proj(3)
                            if b == 0 and st % 2 == 1:
                                pop_qkv_b1(2 if qh == 1 and st >= 8 else 1)
                        for f in av_fifo:
                            f()
                        pending_tails += norm_start(b, qh, [ys[h] for h in range(HPC)])
                for f in pending_tails:
                    f()
                sprinkle_proj(len(proj_ready))
    nc.compile()
    return nc


_CACHE = {}


def _get_nc(with_bias: bool) -> bacc.Bacc:
    if with_bias not in _CACHE:
        _CACHE[with_bias] = build_nc(with_bias)
    return _CACHE[with_bias]


def _prep_inputs(x, w_attn, b_attn, w_proj):
    """Host-side shard + layout prep. Returns per-core in_maps."""
    xf = np.ascontiguousarray(
        np.asarray(x, dtype=np.float32).reshape(TOK, C).T
    ).astype(NPBF16)                                   # x^T [C, TOK]
    w = np.asarray(w_attn, dtype=np.float32)
    ba = np.asarray(b_attn, dtype=np.float32)
    wpj = np.asarray(w_proj, dtype=np.float32)
    scale = 1.0 / math.sqrt(D)
    with_bias = bool(np.any(ba))

    tri_np = np.triu(np.ones((P, P), dtype=np.float32)).astype(NPBF16)
    id_np = np.eye(P, dtype=np.float32).astype(NPBF16)
    ones64_np = np.ones((1, 64), dtype=np.float32).astype(NPBF16)
    ones512_np = np.ones((1, 512), dtype=np.float32).astype(NPBF16)

    in_maps = []
    for c in range(NCORES):
        lo, hi = c * HPC * D, (c + 1) * HPC * D        # 128-wide head slice
        wq = w[:, lo:hi] * scale
        wk = w[:, C + lo:C + hi]
        wv = w[:, 2 * C + lo:2 * C + hi]
        wqkv_c = np.concatenate([wq, wk, wv], axis=1).astype(NPBF16)
        wp_c = np.ascontiguousarray(wpj[lo:hi, :]).astype(NPBF16)
        m = {
            "xt": xf,
            "wqkv": wqkv_c,
            "wp": wp_c,
            "tri": tri_np,
            "ident": id_np,
            "ones64": ones64_np,
        }
        if with_bias:
            bq = ba[lo:hi] * scale
            bk = ba[C + lo:C + hi]
            bv = ba[2 * C + lo:2 * C + hi]
            m["bqkv"] = np.concatenate([bq, bk, bv])[None, :].astype(NPBF16)
            m["ones512"] = ones512_np
        in_maps.append(m)
    return in_maps, with_bias


def _combine(results, b_proj):
    acc = np.zeros((B, C, T), dtype=np.float32)
    for r in results:
        acc += np.asarray(r["outT"], dtype=np.float32)
    out = np.transpose(acc, (0, 2, 1))                 # [B, T, C]
    out = out + np.asarray(b_proj, dtype=np.float32)[None, None, :]
    return np.ascontiguousarray(out.astype(np.float32))


def run(x, w_attn, b_attn, w_proj, b_proj, trace=False, trace_cores=None):
    in_maps, with_bias = _prep_inputs(x, w_attn, b_attn, w_proj)
    nc = _get_nc(with_bias)
    res = run_bass_kernel_spmd(
        nc, in_maps, core_ids=list(range(NCORES)),
        trace=trace, trace_cores=trace_cores,
    )
    return _combine(res.results, b_proj), res


def kernel(x, w_attn, b_attn, w_proj, b_proj):
    out, _ = run(x, w_attn, b_attn, w_proj, b_proj, trace=False)
    return out



# revision 5
# speedup vs baseline: 1.0173x; 1.0173x over previous
"""Causal self-attention Trainium2 kernel (8-core head-parallel), v2.

Full inputs in, full output out. Sharding (unchanged from v1):
  - 16 heads / 8 cores -> 2 heads per core, both batch elems.
  - QKV column-parallel: per-core w_attn slice [1024, 384] (q|k|v 128 each),
    q pre-scaled by 1/sqrt(D).
  - c_proj row-parallel: per-core wp slice [128, 1024]; host sums the 8
    partial [B, C, T] outputs (the all-reduce), transposes, adds bias.

v2 changes vs v1 (257us baseline):
  - QW=512 q-windows; per s-tile step the TWO heads' S matmuls are emitted
    back-to-back into one [128, 2, 512] PSUM slab. Their lhsT base
    partitions (0 / 64) auto-derive tile_position row groups (0,0)/(64,0),
    so the two K=64 matmuls run CONCURRENTLY in the PE array (row tiling).
  - One paired exp per step ([128, 2, w] strided ACT op) instead of two.
  - Normalization with zero ACT/PE work: 1/Z via DVE reciprocal_approx_fast
    on the ones-column row of the AV psum, broadcast across 64 partitions on
    GpSimd (partition_broadcast), then one DVE multiply that also drains the
    y psum into yT.
  - Attention starts right after the first 512-token chunk's QKV (~6us in)
    instead of after all of b0's QKV; all remaining QKV / V-transpose / proj
    work is a dependency-ordered filler queue drained between attention
    steps to keep the PE busy during exp waits.
  - x DMA issued token-chunk-major so chunk 0 lands first.

On-device layout per core (all transposed, zero activations transposes):
  xt [C, TOK=4096] bf16; qkv^T = W^T @ xt -> qkT [128(h*64+d), 2, TOK],
  vT [128, TOK]; V2 [128, 32, 2, 65] PE-transposed v tiles with a ones
  column at col 64 (AV then yields the softmax denominator Z as out row 64).
  S^T[s, q] per head-pair into [128, 2, 512] psum; exp(s-4) -> es bf16;
  causal = only q >= s-tile plus one triangular mask multiply (GpSimd) per
  diagonal tile; AV accumulates y [65, 512] per head over s-tiles;
  yT [128, B, T] = y * (1/Z); out^T[b] = wp^T @ yT per [128, 512] tile.
"""

import math
from collections import deque

import numpy as np
import ml_dtypes

import concourse.bass as bass
from concourse import bacc
import concourse.mybir as mybir
from concourse.tile import TileContext
from concourse.bass_utils import run_bass_kernel_spmd

BF16 = mybir.dt.bfloat16
F32 = mybir.dt.float32
NPBF16 = ml_dtypes.bfloat16

P = 128
B, T, C = 2, 2048, 1024
H, D = 16, 64
NCORES = 8
HPC = H // NCORES          # heads per core
TOK = B * T                # 4096 flattened tokens (b-major)
NCT = C // P               # 8 contraction tiles for the projections
QW = 512                   # q window width
NW = T // QW               # 4 windows per batch elem
EXP_BIAS = -4.0            # exp(s - 4): cancels in normalization, guards tail


def _patch_act_tables():
    """Force exp/ln onto the single table set containing both, avoiding
    mid-stream ACT_TABLE_LOAD switches."""
    import concourse.bacc as bacc_mod
    if getattr(bacc_mod, "_act_tables_patched", False):
        return
    orig = bacc_mod.get_activation_tables
    EXP = mybir.ActivationFunctionType.Exp
    LN = mybir.ActivationFunctionType.Ln

    def patched(arch):
        t = orig(arch)
        if any(EXP in f and LN in f for f in t.values()):
            for name, fns in t.items():
                if "natural_log_exp" not in name and (EXP in fns or LN in fns):
                    t[name] = fns - {EXP, LN}
        return t

    bacc_mod.get_activation_tables = patched
    bacc_mod._act_tables_patched = True


def build_nc(with_bias: bool) -> bacc.Bacc:
    _patch_act_tables()
    nc = bacc.Bacc(None, target_bir_lowering=False)

    xt = nc.dram_tensor("xt", [C, TOK], BF16, kind="ExternalInput")
    wqkv = nc.dram_tensor("wqkv", [C, 3 * P], BF16, kind="ExternalInput")
    wp = nc.dram_tensor("wp", [P, C], BF16, kind="ExternalInput")
    tri = nc.dram_tensor("tri", [P, P], BF16, kind="ExternalInput")
    ident = nc.dram_tensor("ident", [P, P], BF16, kind="ExternalInput")
    if with_bias:
        bqkv = nc.dram_tensor("bqkv", [1, 3 * P], BF16, kind="ExternalInput")
        ones512 = nc.dram_tensor("ones512", [1, 512], BF16, kind="ExternalInput")
    outT = nc.dram_tensor("outT", [B, C, T], BF16, kind="ExternalOutput")

    EXP = mybir.ActivationFunctionType.Exp

    with TileContext(nc) as tc:
        with (
            tc.tile_pool(name="consts", bufs=1) as consts,
            tc.tile_pool(name="px", bufs=1) as px,
            tc.tile_pool(name="pqkv", bufs=1) as pqkv,
            tc.tile_pool(name="py", bufs=1) as py,
            tc.tile_pool(name="sbw", bufs=1) as sbw,
            tc.tile_pool(name="ps_sp", bufs=2, space="PSUM") as ps_sp,
            tc.tile_pool(name="ps_y", bufs=2, space="PSUM") as ps_y,
            tc.tile_pool(name="ps_misc", bufs=2, space="PSUM") as ps_misc,
        ):
            # ---- constant loads (small, land first) ----
            tri_sb = consts.tile([P, P], BF16)
            nc.sync.dma_start(tri_sb, tri[:, :])
            ident_sb = consts.tile([P, P], BF16)
            nc.sync.dma_start(ident_sb, ident[:, :])
            wqkv_sb = consts.tile([P, NCT, 3 * P], BF16)
            for ct in range(NCT):
                nc.sync.dma_start(wqkv_sb[:, ct, :], wqkv[ct * P:(ct + 1) * P, :])
            wp_sb = consts.tile([P, C], BF16)
            nc.sync.dma_start(wp_sb, wp[:, :])
            expb = consts.tile([P, 1], F32)
            nc.vector.memset(expb, EXP_BIAS)
            if with_bias:
                bqkv_sb = consts.tile([1, 3 * P], BF16)
                nc.sync.dma_start(bqkv_sb, bqkv[:, :])
                ones512_sb = consts.tile([1, 512], BF16)
                nc.sync.dma_start(ones512_sb, ones512[:, :])

            # x, token-chunk-major so chunk 0 is available first
            xt_sb = px.tile([P, NCT, TOK], BF16)
            for tcc in range(8):
                for ct in range(NCT):
                    nc.sync.dma_start(
                        xt_sb[:, ct, tcc * 512:(tcc + 1) * 512],
                        xt[ct * P:(ct + 1) * P, tcc * 512:(tcc + 1) * 512])

            # big SBUF slabs
            qkT = pqkv.tile([P, 2, TOK], BF16)      # rows h*64+d; dim1 0=q 1=k
            vT = pqkv.tile([P, TOK], BF16)
            V2 = pqkv.tile([P, TOK // P, HPC, 65], BF16)
            nc.vector.memset(V2, 1.0)               # col 64 = ones column
            yT = py.tile([P, B, T], BF16)

            # ---- unit emitters ----
            copy_eng = [0]  # alternate psum->sbuf copies between DVE and ACT

            def copy_out(dst, src, dve_frac=2):
                """dve_frac of 3 copies go to DVE, rest to ACT."""
                copy_eng[0] = (copy_eng[0] + 1) % 3
                if copy_eng[0] < dve_frac:
                    nc.vector.tensor_copy(dst, src)
                else:
                    nc.scalar.copy(dst, src)

            def emit_qkv_unit(tcc, ft):
                pq = ps_misc.tile([P, 512], F32, tag="misc",
                                  name=f"pq_{tcc}_{ft}")
                for ct in range(NCT):
                    nc.tensor.matmul(
                        pq,
                        wqkv_sb[:, ct, ft * P:(ft + 1) * P],
                        xt_sb[:, ct, tcc * 512:(tcc + 1) * 512],
                        start=(ct == 0),
                        stop=(ct == NCT - 1 and not with_bias),
                    )
                if with_bias:
                    nc.tensor.matmul(
                        pq,
                        bqkv_sb[0:1, ft * P:(ft + 1) * P],
                        ones512_sb[0:1, :],
                        start=False, stop=True,
                    )
                if ft < 2:
                    copy_out(qkT[:, ft, tcc * 512:(tcc + 1) * 512], pq)
                else:
                    copy_out(vT[:, tcc * 512:(tcc + 1) * 512], pq)

            def emit_vtrans_unit(sg):  # sg = global s-tile 0..31
                pt = ps_misc.tile([P, P], BF16, tag="misc", name=f"pt_{sg}")
                nc.tensor.transpose(
                    pt, vT[:, sg * P:(sg + 1) * P], ident_sb)
                # cols 0:64 -> head0 d, 64:128 -> head1 d, in one copy
                nc.vector.tensor_copy(
                    V2[:, sg, :, 0:64],
                    pt[:, :].rearrange("p (h d) -> p h d", h=2),
                )

            def emit_proj_unit(b, qh, of):
                po = ps_misc.tile([P, 512], F32, tag="misc",
                                  name=f"po_{b}_{qh}_{of}")
                nc.tensor.matmul(
                    po,
                    wp_sb[:, of * P:(of + 1) * P],
                    yT[:, b, qh * QW:(qh + 1) * QW],
                    start=True, stop=True,
                )
                ot = sbw.tile([P, 512], BF16, tag="ot", bufs=6,
                              name=f"ot_{b}_{qh}_{of}")
                copy_out(ot, po)
                nc.sync.dma_start(
                    outT[b, of * P:(of + 1) * P, qh * QW:(qh + 1) * QW], ot)

            filler = deque()

            def pop_filler(n):
                for _ in range(min(n, len(filler))):
                    f, *a = filler.popleft()
                    f(*a)

            def force_units(pred):
                """Emit every queued unit matching pred (dependency order is
                preserved because filler is popped front-first)."""
                keep = deque()
                while filler:
                    item = filler.popleft()
                    if pred(item):
                        f, *a = item
                        f(*a)
                    else:
                        keep.append(item)
                filler.extend(keep)

            # ---- attention window ----
            def window(b, qh):
                qbase = qh * QW
                n_st = (qbase + QW) // P
                y0 = ps_y.tile([65, QW], F32, tag="y", name=f"y0_{b}_{qh}")
                y1 = ps_y.tile([65, QW], F32, tag="y", name=f"y1_{b}_{qh}")
                ys = (y0, y1)
                av_fifo = deque()
                for st in range(n_st):
                    s0 = st * P
                    qa = max(qbase, s0)
                    off = qa - qbase
                    w = QW - off
                    sp = ps_sp.tile([P, HPC, QW], F32, tag="sp",
                                    name=f"sp_{b}_{qh}_{st}")
                    for h in range(HPC):
                        nc.tensor.matmul(
                            sp[:, h, 0:w],
                            qkT[64 * h:64 * h + 64, 1,
                                b * T + s0:b * T + s0 + P],
                            qkT[64 * h:64 * h + 64, 0,
                                b * T + qa:b * T + qa + w],
                            start=True, stop=True,
                        )
                    es = sbw.tile([P, HPC, QW], BF16, tag="es", bufs=6,
                                  name=f"es_{b}_{qh}_{st}")
                    nc.scalar.activation(
                        es[:, :, 0:w], sp[:, :, 0:w], EXP, bias=expb)
                    if s0 >= qbase:
                        for h in range(HPC):
                            nc.gpsimd.tensor_mul(
                                es[:, h, 0:P], es[:, h, 0:P], tri_sb)

                    for h in range(HPC):
                        def av(h=h, es=es, off=off, w=w, st=st):
                            nc.tensor.matmul(
                                ys[h][0:65, off:off + w],
                                V2[:, b * (T // P) + st, h, :],
                                es[:, h, 0:w],
                                start=(st == 0),
                                stop=(st == n_st - 1),
                            )
                        av_fifo.append(av)
                    while len(av_fifo) > 4:
                        av_fifo.popleft()()
                    pop_filler(1)
                while av_fifo:
                    av_fifo.popleft()()
                # normalization: r = 1/Z on DVE, broadcast on GpSimd,
                # multiply-drain on DVE
                for h in range(HPC):
                    zrow = sbw.tile([1, QW], F32, tag="zrow", bufs=4,
                                    name=f"zrow_{b}_{qh}_{h}")
                    # ACT copy: moves Z from psum partition 64 to partition 0
                    # (custom DVE ops cannot cross partitions)
                    nc.scalar.copy(zrow, ys[h][64:65, 0:QW])
                    rr = sbw.tile([1, QW], F32, tag="rr", bufs=4,
                                  name=f"rr_{b}_{qh}_{h}")
                    nc.vector.reciprocal_approx_fast(rr, zrow)
                    rbh = sbw.tile([64, QW], F32, tag="rb", bufs=4,
                                   name=f"rb_{b}_{qh}_{h}")
                    nc.gpsimd.partition_broadcast(rbh, rr, channels=64)
                    nc.vector.tensor_mul(
                        yT[h * 64:(h + 1) * 64, b, qbase:qbase + QW],
                        ys[h][0:64, 0:QW],
                        rbh,
                    )
                for of in range(NCT):
                    filler.append((emit_proj_unit, b, qh, of))

            # ---- HAM warm-up while first DMAs land ----
            scratch = ps_misc.tile([P, 512], F32, tag="misc")
            for _ in range(12):
                nc.tensor.matmul(
                    scratch[:, 0:384],
                    wqkv_sb[:, 0, 0:P],
                    wqkv_sb[:, 0, :],
                    start=True, stop=True,
                )

            # ---- static schedule ----
            # phase A: QKV + v-transposes for token chunk 0 (tokens 0:512)
            for ft in (0, 1, 2):
                emit_qkv_unit(0, ft)
            for sg in range(4):
                emit_vtrans_unit(sg)

            # filler queue in dependency-safe order
            for tcc in range(1, 4):
                for ft in (0, 1, 2):
                    filler.append((emit_qkv_unit, tcc, ft))
                for sg in range(4 * tcc, 4 * tcc + 4):
                    filler.append((emit_vtrans_unit, sg))
            for tcc in range(4, 8):
                for ft in (2, 0, 1):
                    filler.append((emit_qkv_unit, tcc, ft))
                for sg in range(4 * tcc, 4 * tcc + 4):
                    filler.append((emit_vtrans_unit, sg))

            def need_for(b, qh):
                """Units that must be emitted before window (b, qh)."""
                tcc_max = b * 4 + qh
                sg_max = b * (T // P) + (qh + 1) * 4 - 1

                def pred(item):
                    f = item[0]
                    if f is emit_qkv_unit:
                        return item[1] <= tcc_max
                    if f is emit_vtrans_unit:
                        return item[1] <= sg_max
                    return False
                return pred

            for b in range(B):
                for qh in range(NW):
                    force_units(need_for(b, qh))
                    window(b, qh)
            pop_filler(len(filler))
    nc.compile()
    return nc


_CACHE = {}


def _get_nc(with_bias: bool) -> bacc.Bacc:
    if with_bias not in _CACHE:
        _CACHE[with_bias] = build_nc(with_bias)
    return _CACHE[with_bias]


def _prep_inputs(x, w_attn, b_attn, w_proj):
    """Host-side shard + layout prep. Returns per-core in_maps."""
    xf = np.ascontiguousarray(
        np.asarray(x, dtype=np.float32).reshape(TOK, C).T
    ).astype(NPBF16)                                   # x^T [C, TOK]
    w = np.asarray(w_attn, dtype=np.float32)
    ba = np.asarray(b_attn, dtype=np.float32)
    wpj = np.asarray(w_proj, dtype=np.float32)
    scale = 1.0 / math.sqrt(D)
    with_bias = bool(np.any(ba))

    tri_np = np.triu(np.ones((P, P), dtype=np.float32)).astype(NPBF16)
    id_np = np.eye(P, dtype=np.float32).astype(NPBF16)
    ones512_np = np.ones((1, 512), dtype=np.float32).astype(NPBF16)

    in_maps = []
    for c in range(NCORES):
        lo, hi = c * HPC * D, (c + 1) * HPC * D        # 128-wide head slice
        wq = w[:, lo:hi] * scale
        wk = w[:, C + lo:C + hi]
        wv = w[:, 2 * C + lo:2 * C + hi]
        wqkv_c = np.concatenate([wq, wk, wv], axis=1).astype(NPBF16)
        wp_c = np.ascontiguousarray(wpj[lo:hi, :]).astype(NPBF16)
        m = {
            "xt": xf,
            "wqkv": wqkv_c,
            "wp": wp_c,
            "tri": tri_np,
            "ident": id_np,
        }
        if with_bias:
            bq = ba[lo:hi] * scale
            bk = ba[C + lo:C + hi]
            bv = ba[2 * C + lo:2 * C + hi]
            m["bqkv"] = np.concatenate([bq, bk, bv])[None, :].astype(NPBF16)
            m["ones512"] = ones512_np
        in_maps.append(m)
    return in_maps, with_bias


def _combine(results, b_proj):
    acc = np.zeros((B, C, T), dtype=np.float32)
    for r in results:
        acc += np.asarray(r["outT"], dtype=np.float32)
    out = np.transpose(acc, (0, 2, 1))                 # [B, T, C]
    out = out + np.asarray(b_proj, dtype=np.float32)[None, None, :]
    return np.ascontiguousarray(out.astype(np.float32))


def run(x, w_attn, b_attn, w_proj, b_proj, trace=False, trace_cores=None):
    in_maps, with_bias = _prep_inputs(x, w_attn, b_attn, w_proj)
    nc = _get_nc(with_bias)
    res = run_bass_kernel_spmd(
        nc, in_maps, core_ids=list(range(NCORES)),
        trace=trace, trace_cores=trace_cores,
    )
    return _combine(res.results, b_proj), res


def kernel(x, w_attn, b_attn, w_proj, b_proj):
    out, _ = run(x, w_attn, b_attn, w_proj, b_proj, trace=False)
    return out


# revision 7
# speedup vs baseline: 1.1674x; 1.1475x over previous
"""Causal self-attention Trainium2 kernel (8-core head-parallel), v2.

Full inputs in, full output out. Sharding (unchanged from v1):
  - 16 heads / 8 cores -> 2 heads per core, both batch elems.
  - QKV column-parallel: per-core w_attn slice [1024, 384] (q|k|v 128 each),
    q pre-scaled by 1/sqrt(D).
  - c_proj row-parallel: per-core wp slice [128, 1024]; host sums the 8
    partial [B, C, T] outputs (the all-reduce), transposes, adds bias.

v2 changes vs v1 (257us baseline):
  - QW=512 q-windows; per s-tile step the TWO heads' S matmuls are emitted
    back-to-back into one [128, 2, 512] PSUM slab. Their lhsT base
    partitions (0 / 64) auto-derive tile_position row groups (0,0)/(64,0),
    so the two K=64 matmuls run CONCURRENTLY in the PE array (row tiling).
  - One paired exp per step ([128, 2, w] strided ACT op) instead of two.
  - Normalization with zero ACT/PE work: 1/Z via DVE reciprocal_approx_fast
    on the ones-column row of the AV psum, broadcast across 64 partitions on
    GpSimd (partition_broadcast), then one DVE multiply that also drains the
    y psum into yT.
  - Attention starts right after the first 512-token chunk's QKV (~6us in)
    instead of after all of b0's QKV; all remaining QKV / V-transpose / proj
    work is a dependency-ordered filler queue drained between attention
    steps to keep the PE busy during exp waits.
  - x DMA issued token-chunk-major so chunk 0 lands first.

On-device layout per core (all transposed, zero activations transposes):
  xt [C, TOK=4096] bf16; qkv^T = W^T @ xt -> qkT [128(h*64+d), 2, TOK],
  vT [128, TOK]; V2 [128, 32, 2, 65] PE-transposed v tiles with a ones
  column at col 64 (AV then yields the softmax denominator Z as out row 64).
  S^T[s, q] per head-pair into [128, 2, 512] psum; exp(s-4) -> es bf16;
  causal = only q >= s-tile plus one triangular mask multiply (GpSimd) per
  diagonal tile; AV accumulates y [65, 512] per head over s-tiles;
  yT [128, B, T] = y * (1/Z); out^T[b] = wp^T @ yT per [128, 512] tile.
"""

import math
from collections import deque

import numpy as np
import ml_dtypes

import concourse.bass as bass
from concourse import bacc
import concourse.mybir as mybir
from concourse.tile import TileContext
from concourse.bass_utils import run_bass_kernel_spmd

BF16 = mybir.dt.bfloat16
F32 = mybir.dt.float32
NPBF16 = ml_dtypes.bfloat16

P = 128
B, T, C = 2, 2048, 1024
H, D = 16, 64
NCORES = 8
HPC = H // NCORES          # heads per core
TOK = B * T                # 4096 flattened tokens (b-major)
NCT = C // P               # 8 contraction tiles for the projections
QW = 512                   # q window width
NW = T // QW               # 4 windows per batch elem
EXP_BIAS = -4.0            # exp(s - 4): cancels in normalization, guards tail


def _patch_act_tables():
    """Force exp/ln onto the single table set containing both, avoiding
    mid-stream ACT_TABLE_LOAD switches."""
    import concourse.bacc as bacc_mod
    if getattr(bacc_mod, "_act_tables_patched", False):
        return
    orig = bacc_mod.get_activation_tables
    EXP = mybir.ActivationFunctionType.Exp
    LN = mybir.ActivationFunctionType.Ln

    def patched(arch):
        t = orig(arch)
        if any(EXP in f and LN in f for f in t.values()):
            for name, fns in t.items():
                if "natural_log_exp" not in name and (EXP in fns or LN in fns):
                    t[name] = fns - {EXP, LN}
        return t

    bacc_mod.get_activation_tables = patched
    bacc_mod._act_tables_patched = True


def build_nc(with_bias: bool) -> bacc.Bacc:
    _patch_act_tables()
    nc = bacc.Bacc(None, target_bir_lowering=False)

    xt = nc.dram_tensor("xt", [C, TOK], BF16, kind="ExternalInput")
    wqkv = nc.dram_tensor("wqkv", [C, 3 * P], BF16, kind="ExternalInput")
    wp = nc.dram_tensor("wp", [P, C], BF16, kind="ExternalInput")
    tri = nc.dram_tensor("tri", [P, P], BF16, kind="ExternalInput")
    ident = nc.dram_tensor("ident", [P, P], BF16, kind="ExternalInput")
    if with_bias:
        bqkv = nc.dram_tensor("bqkv", [1, 3 * P], BF16, kind="ExternalInput")
        ones512 = nc.dram_tensor("ones512", [1, 512], BF16, kind="ExternalInput")
    outT = nc.dram_tensor("outT", [B, C, T], BF16, kind="ExternalOutput")

    EXP = mybir.ActivationFunctionType.Exp

    with TileContext(nc) as tc:
        with (
            tc.tile_pool(name="consts", bufs=1) as consts,
            tc.tile_pool(name="px", bufs=1) as px,
            tc.tile_pool(name="pqkv", bufs=1) as pqkv,
            tc.tile_pool(name="py", bufs=1) as py,
            tc.tile_pool(name="sbw", bufs=1) as sbw,
            tc.tile_pool(name="ps_sp", bufs=2, space="PSUM") as ps_sp,
            tc.tile_pool(name="ps_y", bufs=2, space="PSUM") as ps_y,
            tc.tile_pool(name="ps_misc", bufs=2, space="PSUM") as ps_misc,
        ):
            # ---- constant loads (small, land first) ----
            tri_sb = consts.tile([P, P], BF16)
            nc.sync.dma_start(tri_sb, tri[:, :])
            ident_sb = consts.tile([P, P], BF16)
            nc.sync.dma_start(ident_sb, ident[:, :])
            wqkv_sb = consts.tile([P, NCT, 3 * P], BF16)
            for ct in range(NCT):
                nc.sync.dma_start(wqkv_sb[:, ct, :], wqkv[ct * P:(ct + 1) * P, :])
            wp_sb = consts.tile([P, C], BF16)
            nc.sync.dma_start(wp_sb, wp[:, :])
            expb = consts.tile([P, 1], F32)
            nc.vector.memset(expb, EXP_BIAS)
            if with_bias:
                bqkv_sb = consts.tile([1, 3 * P], BF16)
                nc.sync.dma_start(bqkv_sb, bqkv[:, :])
                ones512_sb = consts.tile([1, 512], BF16)
                nc.sync.dma_start(ones512_sb, ones512[:, :])

            # x, token-chunk-major so chunk 0 is available first
            xt_sb = px.tile([P, NCT, TOK], BF16)
            for tcc in range(8):
                for ct in range(NCT):
                    nc.sync.dma_start(
                        xt_sb[:, ct, tcc * 512:(tcc + 1) * 512],
                        xt[ct * P:(ct + 1) * P, tcc * 512:(tcc + 1) * 512])

            # big SBUF slabs
            qkT = pqkv.tile([P, 2, TOK], BF16)      # rows h*64+d; dim1 0=q 1=k
            vT = pqkv.tile([P, TOK], BF16)
            V2 = pqkv.tile([P, TOK // P, HPC, 65], BF16)
            nc.vector.memset(V2, 1.0)               # col 64 = ones column
            yT = py.tile([P, B, T], BF16)

            # ---- unit emitters ----
            copy_eng = [0]  # alternate psum->sbuf copies between DVE and ACT

            def copy_out(dst, src, dve_frac=2):
                """dve_frac of 3 copies go to DVE, rest to ACT."""
                copy_eng[0] = (copy_eng[0] + 1) % 3
                if copy_eng[0] < dve_frac:
                    nc.vector.tensor_copy(dst, src)
                else:
                    nc.scalar.copy(dst, src)

            qkv_pending = {}

            def emit_qkv_half(tcc, ft, half):
                """First half allocates the psum tile and does ct 0-3; the
                second half finishes ct 4-7 (+bias) and drains. Split so the
                filler granularity stays ~1us of PE work."""
                if half == 0:
                    pq = ps_misc.tile([P, 512], F32, tag="misc",
                                      name=f"pq_{tcc}_{ft}")
                    qkv_pending[(tcc, ft)] = pq
                    cts = range(0, NCT // 2)
                else:
                    pq = qkv_pending.pop((tcc, ft))
                    cts = range(NCT // 2, NCT)
                for ct in cts:
                    nc.tensor.matmul(
                        pq,
                        wqkv_sb[:, ct, ft * P:(ft + 1) * P],
                        xt_sb[:, ct, tcc * 512:(tcc + 1) * 512],
                        start=(ct == 0),
                        stop=(ct == NCT - 1 and not with_bias),
                    )
                if half == 0:
                    return
                if with_bias:
                    nc.tensor.matmul(
                        pq,
                        bqkv_sb[0:1, ft * P:(ft + 1) * P],
                        ones512_sb[0:1, :],
                        start=False, stop=True,
                    )
                if ft < 2:
                    copy_out(qkT[:, ft, tcc * 512:(tcc + 1) * 512], pq)
                else:
                    copy_out(vT[:, tcc * 512:(tcc + 1) * 512], pq)

            def emit_qkv_unit(tcc, ft):
                emit_qkv_half(tcc, ft, 0)
                emit_qkv_half(tcc, ft, 1)

            def emit_vtrans_unit(sg):  # sg = global s-tile 0..31
                pt = ps_misc.tile([P, P], BF16, tag="misc", name=f"pt_{sg}")
                nc.tensor.transpose(
                    pt, vT[:, sg * P:(sg + 1) * P], ident_sb)
                # cols 0:64 -> head0 d, 64:128 -> head1 d, in one copy
                nc.vector.tensor_copy(
                    V2[:, sg, :, 0:64],
                    pt[:, :].rearrange("p (h d) -> p h d", h=2),
                )

            def emit_proj_unit(b, qh, of):
                po = ps_misc.tile([P, 512], F32, tag="misc",
                                  name=f"po_{b}_{qh}_{of}")
                nc.tensor.matmul(
                    po,
                    wp_sb[:, of * P:(of + 1) * P],
                    yT[:, b, qh * QW:(qh + 1) * QW],
                    start=True, stop=True,
                )
                ot = sbw.tile([P, 512], BF16, tag="ot", bufs=6,
                              name=f"ot_{b}_{qh}_{of}")
                copy_out(ot, po)
                nc.sync.dma_start(
                    outT[b, of * P:(of + 1) * P, qh * QW:(qh + 1) * QW], ot)

            filler = deque()

            def pop_filler(n):
                for _ in range(min(n, len(filler))):
                    f, *a = filler.popleft()
                    f(*a)

            def force_units(pred):
                """Emit every queued unit matching pred (dependency order is
                preserved because filler is popped front-first)."""
                keep = deque()
                while filler:
                    item = filler.popleft()
                    if pred(item):
                        f, *a = item
                        f(*a)
                    else:
                        keep.append(item)
                filler.extend(keep)

            # ---- attention window ----
            def make_norm_tails(b, qh, ys):
                """Normalization chain for a finished window, returned as
                closures drained inside the NEXT window's steps (keeps the
                ACT->DVE->Pool->DVE chain latency off the critical path)."""
                qbase = qh * QW
                state = {}

                def t_recip():
                    for h in range(HPC):
                        zrow = sbw.tile([1, QW], F32, tag="zrow", bufs=4,
                                        name=f"zrow_{b}_{qh}_{h}")
                        # ACT copy: moves Z from psum partition 64 to
                        # partition 0 (custom DVE ops cannot cross partitions)
                        nc.scalar.copy(zrow, ys[h][64:65, 0:QW])
                        rr = sbw.tile([1, QW], F32, tag="rr", bufs=4,
                                      name=f"rr_{b}_{qh}_{h}")
                        nc.vector.reciprocal_approx_fast(rr, zrow)
                        state[h] = rr

                def t_bcast():
                    for h in range(HPC):
                        rbh = sbw.tile([64, QW], F32, tag="rb", bufs=4,
                                       name=f"rb_{b}_{qh}_{h}")
                        nc.gpsimd.partition_broadcast(rbh, state[h],
                                                      channels=64)
                        state[h] = rbh

                def t_mult():
                    for h in range(HPC):
                        nc.vector.tensor_mul(
                            yT[h * 64:(h + 1) * 64, b, qbase:qbase + QW],
                            ys[h][0:64, 0:QW],
                            state[h],
                        )
                    for of in range(NCT):
                        filler.append((emit_proj_unit, b, qh, of))

                return [t_recip, t_bcast, t_mult]

            pending_tails = []

            def window(b, qh):
                qbase = qh * QW
                n_st = (qbase + QW) // P
                y0 = ps_y.tile([65, QW], F32, tag="y", name=f"y0_{b}_{qh}")
                y1 = ps_y.tile([65, QW], F32, tag="y", name=f"y1_{b}_{qh}")
                ys = (y0, y1)
                av_fifo = deque()
                for st in range(n_st):
                    s0 = st * P
                    qa = max(qbase, s0)
                    off = qa - qbase
                    w = QW - off
                    sp = ps_sp.tile([P, HPC, QW], F32, tag="sp",
                                    name=f"sp_{b}_{qh}_{st}")
                    for h in range(HPC):
                        nc.tensor.matmul(
                            sp[:, h, 0:w],
                            qkT[64 * h:64 * h + 64, 1,
                                b * T + s0:b * T + s0 + P],
                            qkT[64 * h:64 * h + 64, 0,
                                b * T + qa:b * T + qa + w],
                            start=True, stop=True,
                        )
                    es = sbw.tile([P, HPC, QW], BF16, tag="es", bufs=8,
                                  name=f"es_{b}_{qh}_{st}")
                    nc.scalar.activation(
                        es[:, :, 0:w], sp[:, :, 0:w], EXP, bias=expb)
                    if s0 >= qbase:
                        for h in range(HPC):
                            nc.vector.tensor_mul(
                                es[:, h, 0:P], es[:, h, 0:P], tri_sb)

                    for h in range(HPC):
                        def av(h=h, es=es, off=off, w=w, st=st):
                            nc.tensor.matmul(
                                ys[h][0:65, off:off + w],
                                V2[:, b * (T // P) + st, h, :],
                                es[:, h, 0:w],
                                start=(st == 0),
                                stop=(st == n_st - 1),
                            )
                        av_fifo.append(av)
                    while len(av_fifo) > 4:
                        av_fifo.popleft()()
                    if pending_tails:
                        pending_tails.pop(0)()
                    else:
                        pop_filler(1)
                while av_fifo:
                    av_fifo.popleft()()
                pending_tails.extend(make_norm_tails(b, qh, ys))

            # ---- HAM warm-up while first DMAs land ----
            scratch = ps_misc.tile([P, 512], F32, tag="misc")
            for _ in range(12):
                nc.tensor.matmul(
                    scratch[:, 0:384],
                    wqkv_sb[:, 0, 0:P],
                    wqkv_sb[:, 0, :],
                    start=True, stop=True,
                )

            # ---- static schedule ----
            # phase A: QKV + v-transposes for token chunk 0 (tokens 0:512)
            for ft in (0, 1, 2):
                emit_qkv_unit(0, ft)
            for sg in range(4):
                emit_vtrans_unit(sg)

            # filler queue in dependency-safe order
            for tcc in range(1, 4):
                for ft in (0, 1, 2):
                    filler.append((emit_qkv_unit, tcc, ft))
                for sg in range(4 * tcc, 4 * tcc + 4):
                    filler.append((emit_vtrans_unit, sg))
            for tcc in range(4, 8):
                for ft in (2, 0, 1):
                    filler.append((emit_qkv_unit, tcc, ft))
                for sg in range(4 * tcc, 4 * tcc + 4):
                    filler.append((emit_vtrans_unit, sg))

            def need_for(b, qh):
                """Units that must be emitted before window (b, qh)."""
                tcc_max = b * 4 + qh
                sg_max = b * (T // P) + (qh + 1) * 4 - 1

                def pred(item):
                    f = item[0]
                    if f is emit_qkv_unit:
                        return item[1] <= tcc_max
                    if f is emit_vtrans_unit:
                        return item[1] <= sg_max
                    return False
                return pred

            for b in range(B):
                for qh in range(NW):
                    force_units(need_for(b, qh))
                    window(b, qh)
            for t in pending_tails:
                t()
            pop_filler(len(filler))
    nc.compile()
    return nc


_CACHE = {}


def _get_nc(with_bias: bool) -> bacc.Bacc:
    if with_bias not in _CACHE:
        _CACHE[with_bias] = build_nc(with_bias)
    return _CACHE[with_bias]


def _prep_inputs(x, w_attn, b_attn, w_proj):
    """Host-side shard + layout prep. Returns per-core in_maps."""
    xf = np.ascontiguousarray(
        np.asarray(x, dtype=np.float32).reshape(TOK, C).T
    ).astype(NPBF16)                                   # x^T [C, TOK]
    w = np.asarray(w_attn, dtype=np.float32)
    ba = np.asarray(b_attn, dtype=np.float32)
    wpj = np.asarray(w_proj, dtype=np.float32)
    scale = 1.0 / math.sqrt(D)
    with_bias = bool(np.any(ba))

    tri_np = np.triu(np.ones((P, P), dtype=np.float32)).astype(NPBF16)
    id_np = np.eye(P, dtype=np.float32).astype(NPBF16)
    ones512_np = np.ones((1, 512), dtype=np.float32).astype(NPBF16)

    in_maps = []
    for c in range(NCORES):
        lo, hi = c * HPC * D, (c + 1) * HPC * D        # 128-wide head slice
        wq = w[:, lo:hi] * scale
        wk = w[:, C + lo:C + hi]
        wv = w[:, 2 * C + lo:2 * C + hi]
        wqkv_c = np.concatenate([wq, wk, wv], axis=1).astype(NPBF16)
        wp_c = np.ascontiguousarray(wpj[lo:hi, :]).astype(NPBF16)
        m = {
            "xt": xf,
            "wqkv": wqkv_c,
            "wp": wp_c,
            "tri": tri_np,
            "ident": id_np,
        }
        if with_bias:
            bq = ba[lo:hi] * scale
            bk = ba[C + lo:C + hi]
            bv = ba[2 * C + lo:2 * C + hi]
            m["bqkv"] = np.concatenate([bq, bk, bv])[None, :].astype(NPBF16)
            m["ones512"] = ones512_np
        in_maps.append(m)
    return in_maps, with_bias


def _combine(results, b_proj):
    acc = np.zeros((B, C, T), dtype=np.float32)
    for r in results:
        acc += np.asarray(r["outT"], dtype=np.float32)
    out = np.transpose(acc, (0, 2, 1))                 # [B, T, C]
    out = out + np.asarray(b_proj, dtype=np.float32)[None, None, :]
    return np.ascontiguousarray(out.astype(np.float32))


def run(x, w_attn, b_attn, w_proj, b_proj, trace=False, trace_cores=None):
    in_maps, with_bias = _prep_inputs(x, w_attn, b_attn, w_proj)
    nc = _get_nc(with_bias)
    res = run_bass_kernel_spmd(
        nc, in_maps, core_ids=list(range(NCORES)),
        trace=trace, trace_cores=trace_cores,
    )
    return _combine(res.results, b_proj), res


def kernel(x, w_attn, b_attn, w_proj, b_proj):
    out, _ = run(x, w_attn, b_attn, w_proj, b_proj, trace=False)
    return out


# revision 8
# speedup vs baseline: 1.4248x; 1.2205x over previous
"""Causal self-attention Trainium2 kernel (8-core head-parallel), v2.

Full inputs in, full output out. Sharding (unchanged from v1):
  - 16 heads / 8 cores -> 2 heads per core, both batch elems.
  - QKV column-parallel: per-core w_attn slice [1024, 384] (q|k|v 128 each),
    q pre-scaled by 1/sqrt(D).
  - c_proj row-parallel: per-core wp slice [128, 1024]; host sums the 8
    partial [B, C, T] outputs (the all-reduce), transposes, adds bias.

v2 changes vs v1 (257us baseline):
  - QW=512 q-windows; per s-tile step the TWO heads' S matmuls are emitted
    back-to-back into one [128, 2, 512] PSUM slab. Their lhsT base
    partitions (0 / 64) auto-derive tile_position row groups (0,0)/(64,0),
    so the two K=64 matmuls run CONCURRENTLY in the PE array (row tiling).
  - One paired exp per step ([128, 2, w] strided ACT op) instead of two.
  - Normalization with zero ACT/PE work: 1/Z via DVE reciprocal_approx_fast
    on the ones-column row of the AV psum, broadcast across 64 partitions on
    GpSimd (partition_broadcast), then one DVE multiply that also drains the
    y psum into yT.
  - Attention starts right after the first 512-token chunk's QKV (~6us in)
    instead of after all of b0's QKV; all remaining QKV / V-transpose / proj
    work is a dependency-ordered filler queue drained between attention
    steps to keep the PE busy during exp waits.
  - x DMA issued token-chunk-major so chunk 0 lands first.

On-device layout per core (all transposed, zero activations transposes):
  xt [C, TOK=4096] bf16; qkv^T = W^T @ xt -> qkT [128(h*64+d), 2, TOK],
  vT [128, TOK]; V2 [128, 32, 2, 65] PE-transposed v tiles with a ones
  column at col 64 (AV then yields the softmax denominator Z as out row 64).
  S^T[s, q] per head-pair into [128, 2, 512] psum; exp(s-4) -> es bf16;
  causal = only q >= s-tile plus one triangular mask multiply (GpSimd) per
  diagonal tile; AV accumulates y [65, 512] per head over s-tiles;
  yT [128, B, T] = y * (1/Z); out^T[b] = wp^T @ yT per [128, 512] tile.
"""

import math
from collections import deque

import numpy as np
import ml_dtypes

import concourse.bass as bass
from concourse import bacc
import concourse.mybir as mybir
from concourse.tile import TileContext
from concourse.bass_utils import run_bass_kernel_spmd

BF16 = mybir.dt.bfloat16
F32 = mybir.dt.float32
NPBF16 = ml_dtypes.bfloat16

P = 128
B, T, C = 2, 2048, 1024
H, D = 16, 64
NCORES = 8
HPC = H // NCORES          # heads per core
TOK = B * T                # 4096 flattened tokens (b-major)
NCT = C // P               # 8 contraction tiles for the projections
QW = 512                   # q window width
NW = T // QW               # 4 windows per batch elem
EXP_BIAS = -4.0            # exp(s - 4): cancels in normalization, guards tail


def _patch_act_tables():
    """Force exp/ln onto the single table set containing both, avoiding
    mid-stream ACT_TABLE_LOAD switches."""
    import concourse.bacc as bacc_mod
    if getattr(bacc_mod, "_act_tables_patched", False):
        return
    orig = bacc_mod.get_activation_tables
    EXP = mybir.ActivationFunctionType.Exp
    LN = mybir.ActivationFunctionType.Ln

    def patched(arch):
        t = orig(arch)
        if any(EXP in f and LN in f for f in t.values()):
            for name, fns in t.items():
                if "natural_log_exp" not in name and (EXP in fns or LN in fns):
                    t[name] = fns - {EXP, LN}
        return t

    bacc_mod.get_activation_tables = patched
    bacc_mod._act_tables_patched = True


def build_nc(with_bias: bool) -> bacc.Bacc:
    _patch_act_tables()
    nc = bacc.Bacc(None, target_bir_lowering=False)

    xt = nc.dram_tensor("xt", [C, TOK], BF16, kind="ExternalInput")
    wqkv = nc.dram_tensor("wqkv", [C, 3 * P], BF16, kind="ExternalInput")
    wp = nc.dram_tensor("wp", [P, C], BF16, kind="ExternalInput")
    trimask = nc.dram_tensor("trimask", [P, P], BF16, kind="ExternalInput")
    ident = nc.dram_tensor("ident", [P, P], BF16, kind="ExternalInput")
    if with_bias:
        bqkv = nc.dram_tensor("bqkv", [1, 3 * P], BF16, kind="ExternalInput")
        ones512 = nc.dram_tensor("ones512", [1, 512], BF16, kind="ExternalInput")
    outT = nc.dram_tensor("outT", [B, C, T], BF16, kind="ExternalOutput")

    EXP = mybir.ActivationFunctionType.Exp

    with TileContext(nc) as tc:
        with (
            tc.tile_pool(name="consts", bufs=1) as consts,
            tc.tile_pool(name="px", bufs=1) as px,
            tc.tile_pool(name="pqkv", bufs=1) as pqkv,
            tc.tile_pool(name="py", bufs=1) as py,
            tc.tile_pool(name="sbw", bufs=1) as sbw,
            tc.tile_pool(name="ps_sp", bufs=2, space="PSUM") as ps_sp,
            tc.tile_pool(name="ps_y", bufs=2, space="PSUM") as ps_y,
            tc.tile_pool(name="ps_misc", bufs=2, space="PSUM") as ps_misc,
        ):
            # ---- constant loads (small, land first) ----
            trimask_sb = consts.tile([P, P], BF16)
            nc.sync.dma_start(trimask_sb, trimask[:, :])
            ident_sb = consts.tile([P, P], BF16)
            nc.sync.dma_start(ident_sb, ident[:, :])
            wqkv_sb = consts.tile([P, NCT, 3 * P], BF16)
            for ct in range(NCT):
                nc.sync.dma_start(wqkv_sb[:, ct, :], wqkv[ct * P:(ct + 1) * P, :])
            wp_sb = consts.tile([P, C], BF16)
            nc.sync.dma_start(wp_sb, wp[:, :])
            expb = consts.tile([P, 1], F32)
            nc.vector.memset(expb, EXP_BIAS)
            if with_bias:
                bqkv_sb = consts.tile([1, 3 * P], BF16)
                nc.sync.dma_start(bqkv_sb, bqkv[:, :])
                ones512_sb = consts.tile([1, 512], BF16)
                nc.sync.dma_start(ones512_sb, ones512[:, :])

            # x, token-chunk-major so chunk 0 is available first
            xt_sb = px.tile([P, NCT, TOK], BF16)
            for tcc in range(8):
                for ct in range(NCT):
                    nc.sync.dma_start(
                        xt_sb[:, ct, tcc * 512:(tcc + 1) * 512],
                        xt[ct * P:(ct + 1) * P, tcc * 512:(tcc + 1) * 512])

            # big SBUF slabs
            qkT = pqkv.tile([P, 2, TOK], BF16)      # rows h*64+d; dim1 0=q 1=k
            vT = pqkv.tile([P, TOK], BF16)
            V2 = pqkv.tile([P, TOK // P, HPC, 65], BF16)
            nc.vector.memset(V2, 1.0)               # col 64 = ones column
            yT = py.tile([P, B, T], BF16)

            # ---- unit emitters ----
            copy_eng = [0]  # alternate ot psum->sbuf copies DVE/ACT

            def copy_out(dst, src, alternate=False):
                copy_eng[0] ^= 1
                if not alternate or copy_eng[0]:
                    nc.vector.tensor_copy(dst, src)
                else:
                    nc.scalar.copy(dst, src)

            qkv_pending = {}

            def emit_qkv_half(tcc, ft, half):
                """First half allocates the psum tile and does ct 0-3; the
                second half finishes ct 4-7 (+bias) and drains. Split so the
                filler granularity stays ~1us of PE work."""
                if half == 0:
                    pq = ps_misc.tile([P, 512], F32, tag="misc",
                                      name=f"pq_{tcc}_{ft}")
                    qkv_pending[(tcc, ft)] = pq
                    cts = range(0, NCT // 2)
                else:
                    pq = qkv_pending.pop((tcc, ft))
                    cts = range(NCT // 2, NCT)
                for ct in cts:
                    nc.tensor.matmul(
                        pq,
                        wqkv_sb[:, ct, ft * P:(ft + 1) * P],
                        xt_sb[:, ct, tcc * 512:(tcc + 1) * 512],
                        start=(ct == 0),
                        stop=(ct == NCT - 1 and not with_bias),
                    )
                if half == 0:
                    return
                if with_bias:
                    nc.tensor.matmul(
                        pq,
                        bqkv_sb[0:1, ft * P:(ft + 1) * P],
                        ones512_sb[0:1, :],
                        start=False, stop=True,
                    )
                if ft < 2:
                    copy_out(qkT[:, ft, tcc * 512:(tcc + 1) * 512], pq)
                else:
                    copy_out(vT[:, tcc * 512:(tcc + 1) * 512], pq)

            def emit_qkv_unit(tcc, ft):
                emit_qkv_half(tcc, ft, 0)
                emit_qkv_half(tcc, ft, 1)

            def emit_vtrans_unit(sg):  # sg = global s-tile 0..31
                pt = ps_misc.tile([P, P], BF16, tag="misc", name=f"pt_{sg}")
                nc.tensor.transpose(
                    pt, vT[:, sg * P:(sg + 1) * P], ident_sb)
                # cols 0:64 -> head0 d, 64:128 -> head1 d, in one copy
                nc.vector.tensor_copy(
                    V2[:, sg, :, 0:64],
                    pt[:, :].rearrange("p (h d) -> p h d", h=2),
                )

            def emit_proj_unit(b, qh, of):
                po = ps_misc.tile([P, 512], F32, tag="misc",
                                  name=f"po_{b}_{qh}_{of}")
                nc.tensor.matmul(
                    po,
                    wp_sb[:, of * P:(of + 1) * P],
                    yT[:, b, qh * QW:(qh + 1) * QW],
                    start=True, stop=True,
                )
                ot = sbw.tile([P, 512], BF16, tag="ot", bufs=6,
                              name=f"ot_{b}_{qh}_{of}")
                copy_out(ot, po, alternate=True)
                nc.sync.dma_start(
                    outT[b, of * P:(of + 1) * P, qh * QW:(qh + 1) * QW], ot)

            filler = deque()

            def pop_filler(n):
                for _ in range(min(n, len(filler))):
                    f, *a = filler.popleft()
                    f(*a)

            def force_units(pred):
                """Emit every queued unit matching pred (dependency order is
                preserved because filler is popped front-first)."""
                keep = deque()
                while filler:
                    item = filler.popleft()
                    if pred(item):
                        f, *a = item
                        f(*a)
                    else:
                        keep.append(item)
                filler.extend(keep)

            # ---- attention window ----
            def make_norm_tails(b, qh, ys):
                """Normalization chain for a finished window, returned as
                closures drained inside the NEXT window's steps (keeps the
                ACT->DVE->Pool->DVE chain latency off the critical path)."""
                qbase = qh * QW
                state = {}

                def t_recip():
                    for h in range(HPC):
                        zrow = sbw.tile([1, QW], F32, tag="zrow", bufs=4,
                                        name=f"zrow_{b}_{qh}_{h}")
                        # moves Z from psum partition 64 to partition 0
                        # (custom DVE ops cannot cross partitions)
                        nc.vector.tensor_copy(zrow, ys[h][64:65, 0:QW])
                        rr = sbw.tile([1, QW], F32, tag="rr", bufs=4,
                                      name=f"rr_{b}_{qh}_{h}")
                        nc.vector.reciprocal_approx_fast(rr, zrow)
                        state[h] = rr

                def t_bcast():
                    for h in range(HPC):
                        rbh = sbw.tile([64, QW], F32, tag="rb", bufs=4,
                                       name=f"rb_{b}_{qh}_{h}")
                        nc.gpsimd.partition_broadcast(rbh, state[h],
                                                      channels=64)
                        state[h] = rbh

                def t_mult():
                    for h in range(HPC):
                        nc.vector.tensor_mul(
                            yT[h * 64:(h + 1) * 64, b, qbase:qbase + QW],
                            ys[h][0:64, 0:QW],
                            state[h],
                        )
                    for of in range(NCT):
                        filler.append((emit_proj_unit, b, qh, of))

                return [t_recip, t_bcast, t_mult]

            pending_tails = []

            def window(b, qh):
                qbase = qh * QW
                n_st = (qbase + QW) // P
                y0 = ps_y.tile([65, QW], F32, tag="y", name=f"y0_{b}_{qh}")
                y1 = ps_y.tile([65, QW], F32, tag="y", name=f"y1_{b}_{qh}")
                ys = (y0, y1)
                av_fifo = deque()
                for st in range(n_st):
                    s0 = st * P
                    qa = max(qbase, s0)
                    off = qa - qbase
                    w = QW - off
                    diag = s0 >= qbase
                    sp = ps_sp.tile([P, HPC, QW], F32, tag="sp",
                                    name=f"sp_{b}_{qh}_{st}")
                    for h in range(HPC):
                        nc.tensor.matmul(
                            sp[:, h, 0:w],
                            qkT[64 * h:64 * h + 64, 1,
                                b * T + s0:b * T + s0 + P],
                            qkT[64 * h:64 * h + 64, 0,
                                b * T + qa:b * T + qa + w],
                            start=True, stop=not diag,
                        )
                    if diag:
                        # causal mask folded into S: add -30 on j < s_local
                        # (ident^T @ trimask accumulates the constant strict
                        # lower-triangular -30 block onto the first 128 cols)
                        for h in range(HPC):
                            nc.tensor.matmul(
                                sp[:, h, 0:P],
                                ident_sb,
                                trimask_sb,
                                start=False, stop=True,
                            )
                    es = sbw.tile([P, HPC, QW], BF16, tag="es", bufs=8,
                                  name=f"es_{b}_{qh}_{st}")
                    nc.scalar.activation(
                        es[:, :, 0:w], sp[:, :, 0:w], EXP, bias=expb)

                    for h in range(HPC):
                        def av(h=h, es=es, off=off, w=w, st=st):
                            nc.tensor.matmul(
                                ys[h][0:65, off:off + w],
                                V2[:, b * (T // P) + st, h, :],
                                es[:, h, 0:w],
                                start=(st == 0),
                                stop=(st == n_st - 1),
                            )
                        av_fifo.append(av)
                    while len(av_fifo) > 6:
                        av_fifo.popleft()()
                    if pending_tails:
                        pending_tails.pop(0)()
                    else:
                        pop_filler(1)
                while av_fifo:
                    av_fifo.popleft()()
                pending_tails.extend(make_norm_tails(b, qh, ys))

            # preload the exp table set during the initial DMA wait
            tbl_warm = consts.tile([1, 1], F32)
            nc.scalar.activation(tbl_warm, expb[0:1, 0:1], EXP)

            # ---- HAM warm-up while first DMAs land ----
            scratch = ps_misc.tile([P, 512], F32, tag="misc")
            for _ in range(12):
                nc.tensor.matmul(
                    scratch[:, 0:384],
                    wqkv_sb[:, 0, 0:P],
                    wqkv_sb[:, 0, :],
                    start=True, stop=True,
                )

            # ---- static schedule ----
            # phase A: QKV + v-transposes for token chunk 0 (tokens 0:512)
            for ft in (0, 1, 2):
                emit_qkv_unit(0, ft)
            for sg in range(4):
                emit_vtrans_unit(sg)

            # filler queue in dependency-safe order
            for tcc in range(1, 4):
                for ft in (0, 1, 2):
                    filler.append((emit_qkv_unit, tcc, ft))
                for sg in range(4 * tcc, 4 * tcc + 4):
                    filler.append((emit_vtrans_unit, sg))
            for tcc in range(4, 8):
                for ft in (2, 0, 1):
                    filler.append((emit_qkv_unit, tcc, ft))
                for sg in range(4 * tcc, 4 * tcc + 4):
                    filler.append((emit_vtrans_unit, sg))

            def need_for(b, qh):
                """Units that must be emitted before window (b, qh)."""
                tcc_max = b * 4 + qh
                sg_max = b * (T // P) + (qh + 1) * 4 - 1

                def pred(item):
                    f = item[0]
                    if f is emit_qkv_unit:
                        return item[1] <= tcc_max
                    if f is emit_vtrans_unit:
                        return item[1] <= sg_max
                    return False
                return pred

            for b in range(B):
                for qh in range(NW):
                    force_units(need_for(b, qh))
                    window(b, qh)
            for t in pending_tails:
                t()
            pop_filler(len(filler))
    nc.compile()
    return nc


_CACHE = {}


def _get_nc(with_bias: bool) -> bacc.Bacc:
    if with_bias not in _CACHE:
        _CACHE[with_bias] = build_nc(with_bias)
    return _CACHE[with_bias]


def _prep_inputs(x, w_attn, b_attn, w_proj):
    """Host-side shard + layout prep. Returns per-core in_maps."""
    xf = np.ascontiguousarray(
        np.asarray(x, dtype=np.float32).reshape(TOK, C).T
    ).astype(NPBF16)                                   # x^T [C, TOK]
    w = np.asarray(w_attn, dtype=np.float32)
    ba = np.asarray(b_attn, dtype=np.float32)
    wpj = np.asarray(w_proj, dtype=np.float32)
    scale = 1.0 / math.sqrt(D)
    with_bias = bool(np.any(ba))

    trimask_np = np.tril(
        np.full((P, P), -30.0, dtype=np.float32), -1).astype(NPBF16)
    id_np = np.eye(P, dtype=np.float32).astype(NPBF16)
    ones512_np = np.ones((1, 512), dtype=np.float32).astype(NPBF16)

    in_maps = []
    for c in range(NCORES):
        lo, hi = c * HPC * D, (c + 1) * HPC * D        # 128-wide head slice
        wq = w[:, lo:hi] * scale
        wk = w[:, C + lo:C + hi]
        wv = w[:, 2 * C + lo:2 * C + hi]
        wqkv_c = np.concatenate([wq, wk, wv], axis=1).astype(NPBF16)
        wp_c = np.ascontiguousarray(wpj[lo:hi, :]).astype(NPBF16)
        m = {
            "xt": xf,
            "wqkv": wqkv_c,
            "wp": wp_c,
            "trimask": trimask_np,
            "ident": id_np,
        }
        if with_bias:
            bq = ba[lo:hi] * scale
            bk = ba[C + lo:C + hi]
            bv = ba[2 * C + lo:2 * C + hi]
            m["bqkv"] = np.concatenate([bq, bk, bv])[None, :].astype(NPBF16)
            m["ones512"] = ones512_np
        in_maps.append(m)
    return in_maps, with_bias


def _combine(results, b_proj):
    acc = np.zeros((B, C, T), dtype=np.float32)
    for r in results:
        acc += np.asarray(r["outT"], dtype=np.float32)
    out = np.transpose(acc, (0, 2, 1))                 # [B, T, C]
    out = out + np.asarray(b_proj, dtype=np.float32)[None, None, :]
    return np.ascontiguousarray(out.astype(np.float32))


def run(x, w_attn, b_attn, w_proj, b_proj, trace=False, trace_cores=None):
    in_maps, with_bias = _prep_inputs(x, w_attn, b_attn, w_proj)
    nc = _get_nc(with_bias)
    res = run_bass_kernel_spmd(
        nc, in_maps, core_ids=list(range(NCORES)),
        trace=trace, trace_cores=trace_cores,
    )
    return _combine(res.results, b_proj), res


def kernel(x, w_attn, b_attn, w_proj, b_proj):
    out, _ = run(x, w_attn, b_attn, w_proj, b_proj, trace=False)
    return out
